# revision 44
# baseline (speedup 1.0000x reference)
"""TransE edge scoring v6: resident col-table + fp8 gathers + DoubleRow.

out[e] = sum_d | h[row[e], d] + g[type[e], d] - h[col[e], d] |

Per core (= row-half x col-slice-of-12500): edges sorted by col. The col
slice table is SBUF-resident fp8 [128, 98, 128]; each 128-edge block's
col term is ONE K=64 matmul: a -1-valued fp8 one-hot selector
(stationary) times the block's 64-col half-tile (moving, at partition
offset 64*((bi//3)%2)) -- no per-edge col DMA or Pool work. Blocks are
grouped 3 per half-tile (capacity 384), so a block's cols always live
in one half-tile; two blocks' K=64 selectors pack into one meta block
on opposite partition halves (halves the sel stream vs K=128). Rows are SWDGE-gathered as fp8 128B payloads from a 256B-stride
table (raw InstDMAGatherAnt; the bass wrapper would force 256B elems --
128B halves the 2x-latency descriptor cost), 1024 idx per instruction
(HW limit, probed), landing INSIDE the chunk's meta tile so the
(gexp, hrow) pair sits at a fixed 27-block stride: one fp8 DoubleRow
matmul (0.5 cyc/row, HW-verified to sum both k-tiles) adds g[type] and
h[row] together. g[type] is host-expanded to an fp8 stream. Meta tile
[128, 63, 128] fp8: blocks 0-11 packed sel (overflow chunk: 0-1 col
idx), 12-35 gexp, 36-38 row idx (int16, bitcast; block 39 zero pad so
the idx copy is 512B/partition, dodging the <512B 2x DMA penalty),
40-63 gather landing. One DVE tensor_reduce(|.|) per half-chunk reads
PSUM into fp16 scores.

Static SPMD layout shared by all 8 cores: 3 blocks per 64-col
half-tile (196 half-tiles -> 588 blocks -> 25 chunks of 24 blocks, 12
dummy), plus one overflow chunk (16 blocks, plain row+col gathers) for
per-half-tile spill (>384 edges). nch = 26 always.

Partial chunks (the 12-real-block chunk 24 and the overflow chunk)
load only their used meta rectangles; the overflow chunk is emitted
after regular chunk 22 so its DMA fills the pipeline drain.

Row-idx meta pieces load via the idle Activation DMA queue so gathers
start without waiting for the sel+gexp copy on the SP queue.

Tail gathers are right-sized (chunk 24: 1024+512 idx; overflow:
1024+768 per side) instead of fixed 1024s.

Cost-model per-core: DMA ~105us busy, Pool 104 (88%), PE 52, DVE 87.
119002 ns total (baseline 255771; 2.15x). Rel err 0.0113 (tol 2e-2).
"""

import sys

sys.path.insert(0, "/opt/trn_rl_repo")

import numpy as np
import ml_dtypes

import concourse.tile as tile
from concourse import ap_utils, bacc, mybir
from concourse.bass_utils import run_bass_kernel_spmd

N_NODES = 50000
N_REL = 500
D = 128
N_EDGES = 600000
NCORES = 8

RH = 25000            # rows per half
CS = 12500            # cols per slice
NT = 98               # col tiles per slice (ceil 12500/128)
BPT = 6               # blocks per tile (regular region)
STRIDE = 256          # fp8 bytes between h-table rows (SWDGE stride enc)
CHUNK = 3072
EB = CHUNK // 128     # 24 blocks per chunk
REG_BLOCKS = NT * BPT            # 588
REG_CH = -(-REG_BLOCKS // EB)    # 25
NCH = REG_CH + 1                 # + overflow chunk
OVF_BLOCKS = 14                  # overflow capacity: 1792 edges
# meta tile block indices ([128, MTB, 128] fp8; first MLOAD loaded via DMA)
# sel is K=64 per block (half-tile selectors); two blocks share one meta
# block on opposite 64-partition halves -> 12 sel blocks per chunk
MT_SEL = 0            # blocks 0..11:  sel (regular) / col idx 0..1 (overflow)
MT_GE = 12            # blocks 12..35: gexp
MT_IDX = 36           # blocks 36..38: row idx int16 wrapped (384B)
# block 39 is zero pad so the idx copy is 512B/partition (no 2x DMA penalty)
MT_HR = 40            # blocks 40..63: row-gather landing (not DMA-loaded)
MLOAD = 40
MTB = 64

F8 = ml_dtypes.float8_e4m3fn

# tuning knobs (read at _build_program time)
RING = 65536          # SWDGE ring bytes (/16 = descriptors in flight)
META_BUFS = 5
OVF_AT = 22           # emit overflow chunk after this many regular chunks

_programs: dict[int, "bacc.Bacc"] = {}


def _wrap16(ids: np.ndarray) -> np.ndarray:
    """[n] -> [128, n//16] int16: idx i at [i%16, i//16], replicated x8."""
    n = len(ids)
    w = ids.reshape(n // 16, 16).T.astype(np.int16)
    return np.ascontiguousarray(np.tile(w, (8, 1)))


def _raw_dma_gather(eng, out_ap, in_ap, idxs_ap, num_idxs, elem_size,
                    elem_step):
    """dma_gather (transpose=False) without the elem%256 restriction."""
    assert idxs_ap.dtype == mybir.dt.int16
    assert in_ap.dtype == out_ap.dtype
    assert ap_utils.ap_is_contiguous(in_ap.ap[1:])
    assert ap_utils.ap_is_contiguous(out_ap.ap[1:])
    assert ap_utils.ap_is_contiguous(idxs_ap.ap[1:])
    assert in_ap.ap[-1][1] == out_ap.ap[-1][1] == elem_size
    assert in_ap.ap[0][0] == elem_step
    assert num_idxs <= 1024  # SWDGE per-instruction limit (HW-verified)
    assert out_ap.ap[0][1] * out_ap.ap[1][1] == num_idxs
    stride_bytes = elem_step * mybir.dt.size(in_ap.dtype)
    stride_bytes_256, rem = divmod(stride_bytes, 256)
    assert rem == 0 and stride_bytes_256 < 256
    _in_ap = eng.lower_ap_dma(in_ap, for_custom_bir_dma=True)
    return eng.add_instruction(
        mybir.InstDMAGatherAnt(
            name=eng.bass.get_next_instruction_name(),
            ins=[*_in_ap, eng.lower_ap(idxs_ap),
                 eng.lower_val_access(eng.to_reg(num_idxs))],
            outs=[eng.lower_ap(out_ap)],
            transpose=False,
            num_idxs=num_idxs,
            elem_size=elem_size,
            stride_bytes_256=stride_bytes_256,
            gen_mode=0,
            single_packet=True,
            queue_num=0,
            sbuf_tokens_per_rank=0,
            sbuf_free_dim_per_rank=0,
            sbuf_free_dim_pad_per_rank=0,
            sbuf_byte_offset=0,
        )
    )


def _tile_of_block(bi: int) -> int:
    return bi // BPT if bi < REG_BLOCKS else NT - 1


def _build_program() -> "bacc.Bacc":
    nc = bacc.Bacc("TRN2", debug=False, dynamic_dma_scratch_size=RING)
    dt = mybir.dt
    DR = mybir.MatmulPerfMode.DoubleRow

    rows8 = nc.declare_dram_parameter("rows8", [RH, STRIDE], dt.float8e4,
                                      isOutput=False)
    cols8 = nc.declare_dram_parameter("cols8", [NT * 128, STRIDE], dt.float8e4,
                                      isOutput=False)
    coltabp = nc.declare_dram_parameter("coltabp", [128, NT, D], dt.float8e4,
                                        isOutput=False)
    ident8 = nc.declare_dram_parameter("ident8", [128, 3, 128], dt.float8e4,
                                       isOutput=False)
    meta = nc.declare_dram_parameter("meta", [NCH, 128, MLOAD, 128],
                                     dt.float8e4, isOutput=False)
    scores = nc.declare_dram_parameter("scores", [NCH, 128, EB], dt.float16,
                                       isOutput=True)

    def idx_ap(mt, blk, j):
        return mt[:, blk + j, :].bitcast(dt.int16)

    def reg_chunk(k, pools, pre_mt=None):
        ip, scp, psm, idt, ct = pools
        if pre_mt is not None:
            mt = pre_mt
        else:
            mt = ip.tile([128, MTB, 128], dt.float8e4, tag="mt")
            nr = min(REG_BLOCKS - k * EB, EB)
            # row idx rides the idle Activation queue so gathers start
            # without waiting for the big sel+gexp copy
            nc.scalar.dma_start(mt[:, MT_IDX:MLOAD, :],
                                meta[k][:, MT_IDX:MLOAD, :])
            if nr == EB:
                nc.sync.dma_start(mt[:, 0:MT_IDX, :],
                                  meta[k][:, 0:MT_IDX, :])
            else:
                nc.sync.dma_start(mt[:, 0:nr // 2, :],
                                  meta[k][:, 0:nr // 2, :])
                nc.sync.dma_start(mt[:, MT_GE:MT_GE + nr, :],
                                  meta[k][:, MT_GE:MT_GE + nr, :])
        nreal = min(REG_BLOCKS - k * EB, EB)
        for j in range(-(-nreal * 128 // 1024)):
            sz = min(1024, nreal * 128 - 1024 * j)
            _raw_dma_gather(
                nc.gpsimd, mt[:, MT_HR + 8 * j:MT_HR + 8 * j + sz // 128, :],
                rows8[:, 0:D], idx_ap(mt, MT_IDX, j),
                num_idxs=sz, elem_size=D, elem_step=STRIDE)
        sct = scp.tile([128, EB], dt.float16, tag="sct")
        for h in range(2):
            if h * (EB // 2) >= nreal:
                break
            ps = psm.tile([128, EB // 2, D], dt.float32, tag="ps")
            for b in range(EB // 2):
                eb = (EB // 2) * h + b
                bi = k * EB + eb
                t = _tile_of_block(bi)
                p0 = 64 * ((bi // 3) % 2)
                m_eb = 3 * (eb // 6) + (eb % 3)
                po = ps[:, b, :]
                nc.tensor.matmul(po, mt[p0:p0 + 64, MT_SEL + m_eb, :],
                                 ct[p0:p0 + 64, t, :],
                                 start=True, stop=False)
                nc.tensor.matmul(
                    po, idt[:, 0:2, :],
                    mt[:, MT_GE + eb:MT_HR + eb + 1:MT_HR - MT_GE, :],
                    start=False, stop=True, perf_mode=DR)
            with nc.allow_low_precision("fp16 scores, tol 2e-2"):
                nc.vector.tensor_reduce(
                    sct[:, (EB // 2) * h:(EB // 2) * (h + 1)],
                    ps[:], axis=mybir.AxisListType.X,
                    op=mybir.AluOpType.add,
                    apply_absolute_value=True)
        nc.sync.dma_start(scores[k], sct[:])

    def ovf_chunk(pools):
        ip, hcp, scp, psm, idt = pools
        mt = ip.tile([128, MTB, 128], dt.float8e4, tag="mt")
        nc.scalar.dma_start(mt[:, 0:2, :], meta[REG_CH][:, 0:2, :])
        nc.scalar.dma_start(mt[:, MT_IDX:MT_IDX + 2, :],
                            meta[REG_CH][:, MT_IDX:MT_IDX + 2, :])
        nc.sync.dma_start(mt[:, MT_GE:MT_GE + OVF_BLOCKS, :],
                          meta[REG_CH][:, MT_GE:MT_GE + OVF_BLOCKS, :])
        hc = hcp.tile([128, OVF_BLOCKS, D], dt.float8e4, tag="hc")
        nslots_o = OVF_BLOCKS * 128
        sizes = [min(1024, nslots_o - 1024 * j)
                 for j in range(-(-nslots_o // 1024))]
        for j, sz in enumerate(sizes):
            _raw_dma_gather(
                nc.gpsimd,
                mt[:, MT_HR + 8 * j:MT_HR + 8 * j + sz // 128, :],
                rows8[:, 0:D], idx_ap(mt, MT_IDX, j),
                num_idxs=sz, elem_size=D, elem_step=STRIDE)
        for j, sz in enumerate(sizes):
            _raw_dma_gather(
                nc.gpsimd, hc[:, 8 * j:8 * j + sz // 128, :],
                cols8[:, 0:D], idx_ap(mt, MT_SEL, j),
                num_idxs=sz, elem_size=D, elem_step=STRIDE)
        sct = scp.tile([128, EB], dt.float16, tag="sct")
        qb = OVF_BLOCKS // 2
        for h in range(2):
            ps = psm.tile([128, EB // 2, D], dt.float32, tag="ps")
            for b in range(qb):
                eb = qb * h + b
                po = ps[:, b, :]
                nc.tensor.matmul(
                    po, idt[:, 0:2, :],
                    mt[:, MT_GE + eb:MT_HR + eb + 1:MT_HR - MT_GE, :],
                    start=True, stop=False, perf_mode=DR)
                nc.tensor.matmul(po, idt[:, 2, :], hc[:, eb, :],
                                 start=False, stop=True)
            with nc.allow_low_precision("fp16 scores, tol 2e-2"):
                nc.vector.tensor_reduce(
                    sct[:, qb * h:qb * (h + 1)],
                    ps[:, 0:qb, :], axis=mybir.AxisListType.X,
                    op=mybir.AluOpType.add,
                    apply_absolute_value=True)
        nc.sync.dma_start(scores[REG_CH], sct[:])

    with tile.TileContext(nc) as tc:
        with tc.tile_pool(name="const", bufs=1) as cp, \
             tc.tile_pool(name="metap", bufs=META_BUFS) as ip, \
             tc.tile_pool(name="hcp", bufs=1) as hcp, \
             tc.tile_pool(name="scp", bufs=3) as scp, \
             tc.tile_pool(name="psm", bufs=2, space="PSUM") as psm:
            idt = cp.tile([128, 3, 128], dt.float8e4, tag="idt")
            nc.sync.dma_start(idt[:], ident8[:])
            ct = cp.tile([128, NT, D], dt.float8e4, tag="ct")
            nc.sync.dma_start(ct[:], coltabp[:])

            for k in range(REG_CH):
                reg_chunk(k, (ip, scp, psm, idt, ct))
                if k == OVF_AT:
                    ovf_chunk((ip, hcp, scp, psm, idt))
    nc.compile()
    return nc


def _prep(h, g, row, col, typ):
    """Host-side shard/sort/pack. Returns (in_maps, placements)."""
    h8p = np.zeros((N_NODES, STRIDE), dtype=F8)
    h8p[:, :D] = h.astype(F8)
    g8 = g.astype(F8)                      # [500, 128]
    eye = np.eye(128, dtype=np.float32)
    ident8 = np.ascontiguousarray(
        np.stack([eye, eye, -eye], axis=1).astype(F8))  # [128, 3, 128]

    core_of = (row // RH) * 4 + (col // CS)
    in_maps, placements = [], []
    for ci in range(NCORES):
        rh, cs = ci // 4, ci % 4
        ids = np.nonzero(core_of == ci)[0]
        cl = col[ids] - CS * cs
        o = np.argsort(cl, kind="stable")
        ids = ids[o]
        cl = cl[o]
        tl = cl // 64                      # half-tile of each edge

        # split regular (first <=384 per half-tile) vs overflow
        counts = np.bincount(tl, minlength=2 * NT)
        starts = np.concatenate([[0], np.cumsum(counts)[:-1]])
        pos_in_tile = np.arange(len(ids)) - starts[tl]
        is_reg = pos_in_tile < 384
        n_ovf = int((~is_reg).sum())
        if n_ovf > OVF_BLOCKS * 128:
            raise RuntimeError(f"core {ci}: overflow {n_ovf}")

        nslots = NCH * CHUNK
        place = np.full(nslots, -1, np.int64)
        slot_reg = tl[is_reg] * 384 + pos_in_tile[is_reg]
        place[slot_reg] = ids[is_reg]
        ovf_base = REG_CH * CHUNK
        place[ovf_base:ovf_base + n_ovf] = ids[~is_reg]

        sslot = np.nonzero(place >= 0)[0]
        eids = place[sslot]
        s_rl = np.zeros(nslots, np.int64)
        s_cl = np.zeros(nslots, np.int64)
        s_ty = np.zeros(nslots, np.int64)
        s_rl[sslot] = row[eids] - RH * rh
        s_cl[sslot] = col[eids] - CS * cs
        s_ty[sslot] = typ[eids]

        # packed meta [NCH, 128, MLOAD*128] bytes
        mu = np.zeros((NCH, 128, MLOAD * 128), np.uint8)
        B_GE, B_IDX = MT_GE * 128, MT_IDX * 128
        # gexp: slot (k, eb, p): [128 p, EB, D] per chunk
        ge = g8[s_ty.reshape(NCH, EB, 128)]           # [NCH, EB, 128, D]
        mu[:, :, B_GE:B_IDX] = ge.transpose(0, 2, 1, 3).reshape(
            NCH, 128, EB * D).view(np.uint8)
        # sel (regular chunks): -1 at [k, cl%128, m_eb*128+pos],
        # m_eb packs two opposite-parity blocks per meta block
        sel = np.zeros((REG_CH, 128, (EB // 2) * 128), np.float16)
        reg_slots = sslot[sslot < REG_CH * CHUNK]
        kk, rem = np.divmod(reg_slots, CHUNK)
        ebb, pp = np.divmod(rem, 128)
        m_ebb = 3 * (ebb // 6) + (ebb % 3)
        sel[kk, s_cl[reg_slots] % 128, m_ebb * 128 + pp] = -1.0
        mu[:REG_CH, :, 0:B_GE] = sel.astype(F8).view(np.uint8)
        # row idx (all chunks) at blocks 48..50
        for k in range(NCH):
            w = _wrap16(s_rl[k * CHUNK:(k + 1) * CHUNK])   # [128, 192]
            mu[k, :, B_IDX:B_IDX + 384] = w.view(np.uint8)
        # overflow col idx at blocks 0..2
        w = _wrap16(s_cl[ovf_base:ovf_base + CHUNK])
        mu[REG_CH, :, 0:384] = w.view(np.uint8)

        # resident col table, partition-major [128, NT, D] fp8
        colslice = np.zeros((NT * 128, D), F8)
        colslice[:CS] = h8p[CS * cs:CS * (cs + 1), :D]
        coltabp = np.ascontiguousarray(
            colslice.reshape(NT, 128, D).transpose(1, 0, 2))
        cols8_tab = np.zeros((NT * 128, STRIDE), F8)
        cols8_tab[:CS] = h8p[CS * cs:CS * (cs + 1)]

        in_maps.append({
            "rows8": np.ascontiguousarray(h8p[RH * rh:RH * (rh + 1)]),
            "cols8": cols8_tab,
            "coltabp": coltabp,
            "ident8": ident8,
            "meta": mu.reshape(NCH, 128, MLOAD, 128).view(F8),
            "scores": np.zeros((NCH, 128, EB), np.float16),
        })
        placements.append(place)
    return in_maps, placements


def kernel(h, g, edge_idx, edge_type):
    h = np.asarray(h, dtype=np.float32)
    g = np.asarray(g, dtype=np.float32)
    row = np.asarray(edge_idx[0]).astype(np.int64)
    col = np.asarray(edge_idx[1]).astype(np.int64)
    typ = np.asarray(edge_type).astype(np.int64)

    in_maps, placements = _prep(h, g, row, col, typ)

    if NCH not in _programs:
        _programs[NCH] = _build_program()
    nc = _programs[NCH]

    results = run_bass_kernel_spmd(nc, in_maps, list(range(NCORES))).results

    out = np.empty(N_EDGES, dtype=np.float32)
    for ci in range(NCORES):
        sc = np.asarray(results[ci]["scores"]).astype(np.float32)
        vals = sc.transpose(0, 2, 1).reshape(-1)      # slot = (k*EB+eb)*128+p
        place = placements[ci]
        m = place >= 0
        out[place[m]] = vals[m]
    return out
